# revision 6
# baseline (speedup 1.0000x reference)
"""Ernie4.5 attention layer on 8 Trainium2 NeuronCores.

Sharding (tensor-parallel over heads, 8-way):
  core i owns query heads {2i, 2i+1} and kv head i//2 (kv groups stay
  aligned with their query heads; kv projections are duplicated on the
  two cores sharing a kv head).
  Every core processes all B*S = 4096 tokens through projections + RoPE +
  attention for its own heads, producing attn^T [256 f, 4096 tok].
  An 8-core AllToAll redistributes head-slices -> token-slices, after
  which core i holds attn^T [2048 f, 512 tok] for global tokens
  [512*i, 512*(i+1)) and computes the o_proj rows for those tokens.
  Host-side gather is a pure concatenation.

Numerics: projections / scores run in float32r (tf32-like, ~1e-4 rel),
softmax probabilities and the PV / o_proj matmuls run in bf16 with fp32
PSUM accumulation.
"""

import math
from contextlib import ExitStack

import numpy as np
import ml_dtypes

import concourse.bass as bass
import concourse.tile as tile
from concourse import bacc, mybir
from concourse.bass_utils import run_bass_kernel_spmd
from concourse.masks import make_identity

HIDDEN = 2048
N_HEADS = 16
N_KV_HEADS = 4
HEAD_DIM = 128
ROPE_BASE = 10000.0
B, S = 2, 2048
T = B * S                    # 4096 global tokens (batch-major)
N_CORES = 8
HEADS_PER_CORE = 2
P = 128
SCALE = 1.0 / math.sqrt(HEAD_DIM)

F32 = mybir.dt.float32
F32R = mybir.dt.float32r
BF16 = mybir.dt.bfloat16

HS = HIDDEN // P             # 16 hidden slices
TB = T // 512                # 8 token blocks of 512
KB = T // P                  # 32 key blocks of 128 (16 per batch)
QC_PER_BATCH = S // P        # 16 query chunks of 128 per batch

SWAP_MASK = [i ^ 1 for i in range(32)]  # pair swap within 32-partition groups


class Ctx:
    """Pools + persistent tiles for one repetition of the program."""

    def __init__(self, tc, es, io):
        nc = tc.nc
        self.tc, self.nc, self.io = tc, nc, io
        pool = lambda name, bufs, **kw: es.enter_context(tc.tile_pool(name=name, bufs=bufs, **kw))
        self.const = pool("const", 1)
        self.wqp = pool("wq", 1)
        self.wkvp = pool("wkv", 1)
        self.xtp = pool("xt", 10)
        self.qtp = pool("qt", 1)
        self.ropep = pool("rope", 4)
        self.vaugp = pool("vaug", 1)
        self.expp = pool("expp", 18)
        self.smallp = pool("small_sb", 4)
        self.attntp = pool("attnt", 16)
        self.wop = pool("wo", 8)
        self.outp = pool("outsb", 3)
        self.psb = pool("ps_big", 5, space="PSUM")
        self.pss = pool("ps_small", 3, space="PSUM")
        self.dram = pool("dram", 1, space="DRAM")

        self.identity = self.const.tile([P, P], BF16, tag="identity", name="identity")
        make_identity(nc, self.identity[:])
        self.tri_sb = self.const.tile([P, P], BF16, tag="tri", name="tri")
        nc.sync.dma_start(out=self.tri_sb[:], in_=io["trimask"][:])
        self.cos_sb = self.const.tile([P, S], F32, tag="cos", name="cos")
        nc.sync.dma_start(out=self.cos_sb[:], in_=io["cosT"][:])
        self.sin_sb = self.const.tile([P, S], F32, tag="sin", name="sin")
        nc.sync.dma_start(out=self.sin_sb[:], in_=io["sinTs"][:])

        self.wq_sb = [self.wqp.tile([P, HEADS_PER_CORE * HEAD_DIM], F32R, tag=f"wq{h}", name=f"wq{h}") for h in range(HS)]
        self.wk_sb = [self.wkvp.tile([P, HEAD_DIM], F32R, tag=f"wk{h}", name=f"wk{h}") for h in range(HS)]
        self.wv_sb = [self.wkvp.tile([P, HEAD_DIM], F32R, tag=f"wv{h}", name=f"wv{h}") for h in range(HS)]
        for h in range(HS):
            nc.sync.dma_start(out=self.wq_sb[h][:], in_=io["wqT"][h * P:(h + 1) * P, :])
            nc.sync.dma_start(out=self.wk_sb[h][:], in_=io["wkT"][h * P:(h + 1) * P, :])
            nc.sync.dma_start(out=self.wv_sb[h][:], in_=io["wvT"][h * P:(h + 1) * P, :])

        # persistent rotated Q^T (per head) and K^T, all [128, 4096] f32r
        self.QT = [self.qtp.tile([P, T], F32R, tag=f"QT{i}", name=f"QT{i}") for i in range(HEADS_PER_CORE)]
        self.KT = self.qtp.tile([P, T], F32R, tag="KT", name="KT")
        # V_aug[kb]: [128 tok, 128 d + ones column] bf16
        self.Vaug = [self.vaugp.tile([P, HEAD_DIM + 1], BF16, tag=f"va{k}", name=f"va{k}") for k in range(KB)]

        # a2a buffers: [8 dst cores x 256 f, 512 tok] bf16
        self.a2a_in = self.dram.tile([N_CORES * HEADS_PER_CORE * HEAD_DIM, 512], BF16, tag="a2a_in", name="a2a_in")
        self.a2a_out = self.dram.tile([N_CORES * HEADS_PER_CORE * HEAD_DIM, 512], BF16, tag="a2a_out", name="a2a_out")


def _rope(cx, ps, dst, tb):
    """dst = psum * cos + pairswap(psum) * signed_sin, written as f32r."""
    nc = cx.nc
    s0 = (tb % (S // 512)) * 512  # seq offset within batch
    shuf = cx.ropep.tile([P, 512], F32, tag="shuf", name="shuf")
    nc.vector.stream_shuffle(shuf[:], ps[:], SWAP_MASK)
    t1 = cx.ropep.tile([P, 512], F32, tag="t1", name="t1")
    nc.vector.tensor_mul(t1[:], ps[:], cx.cos_sb[:, s0:s0 + 512])
    t2 = cx.ropep.tile([P, 512], F32, tag="t2", name="t2")
    nc.vector.tensor_mul(t2[:], shuf[:], cx.sin_sb[:, s0:s0 + 512])
    nc.vector.tensor_add(dst, t1[:], t2[:])


def phase_proj(cx):
    nc = cx.nc
    for tb in range(TB):
        xt = [cx.xtp.tile([P, 512], F32R, tag="xt", name="xt") for _ in range(HS)]
        for h in range(HS):
            nc.sync.dma_start(out=xt[h][:], in_=cx.io["xT"][h * P:(h + 1) * P, tb * 512:(tb + 1) * 512])
        q_ps = [cx.psb.tile([P, 512], F32, tag="big", name="big") for _ in range(HEADS_PER_CORE)]
        k_ps = cx.psb.tile([P, 512], F32, tag="big", name="big")
        vt_ps = cx.psb.tile([P, 512], F32, tag="big", name="big")
        for h in range(HS):
            st, sp = h == 0, h == HS - 1
            for q in range(HEADS_PER_CORE):
                nc.tensor.matmul(q_ps[q][:], cx.wq_sb[h][:, q * P:(q + 1) * P], xt[h][:], start=st, stop=sp)
            nc.tensor.matmul(k_ps[:], cx.wk_sb[h][:], xt[h][:], start=st, stop=sp)
            nc.tensor.matmul(vt_ps[:], cx.wv_sb[h][:], xt[h][:], start=st, stop=sp)
        for q in range(HEADS_PER_CORE):
            _rope(cx, q_ps[q], cx.QT[q][:, tb * 512:(tb + 1) * 512], tb)
        _rope(cx, k_ps, cx.KT[:, tb * 512:(tb + 1) * 512], tb)
        # V: copy to bf16, transpose 128x128 blocks into Vaug ([tok, d])
        vt_sb = cx.smallp.tile([P, 512], BF16, tag="vtsb", name="vtsb")
        nc.vector.tensor_copy(vt_sb[:], vt_ps[:])
        for j in range(4):
            kbi = tb * 4 + j
            vtt = cx.pss.tile([P, P], BF16, tag="small", name="small")
            nc.tensor.transpose(vtt[:], vt_sb[:, j * P:(j + 1) * P], cx.identity[:])
            nc.vector.tensor_copy(cx.Vaug[kbi][:, 0:HEAD_DIM], vtt[:])
            nc.vector.memset(cx.Vaug[kbi][:, HEAD_DIM:HEAD_DIM + 1], 1.0)


def phase_attention(cx):
    nc = cx.nc
    for hq in range(HEADS_PER_CORE):
        for b in range(B):
            kb0 = b * (S // P)      # first key block of this batch
            q0 = b * S              # first token of this batch
            for qt in range(S // 512):
                i0 = qt * 4         # first q chunk (of 128) in this tile
                ep = []
                for kb in range(i0 + 4):
                    st_ps = cx.psb.tile([P, 512], F32, tag="big", name="big")
                    nc.tensor.matmul(
                        st_ps[:],
                        cx.KT[:, (kb0 + kb) * P:(kb0 + kb + 1) * P],
                        cx.QT[hq][:, q0 + qt * 512:q0 + (qt + 1) * 512],
                        start=True, stop=True,
                    )
                    e = cx.expp.tile([P, 512], BF16, tag="expp", name="expp")
                    nc.scalar.activation(e[:], st_ps[:], mybir.ActivationFunctionType.Exp, scale=SCALE)
                    if kb >= i0:
                        m = kb - i0
                        if m > 0:
                            nc.vector.memset(e[:, 0:m * P], 0.0)
                        nc.vector.tensor_mul(e[:, m * P:(m + 1) * P], e[:, m * P:(m + 1) * P], cx.tri_sb[:])
                    ep.append(e)
                for c in range(4):
                    qb = i0 + c
                    oaug = cx.pss.tile([P, HEAD_DIM + 1], F32, tag="small", name="small")
                    for kb in range(qb + 1):
                        nc.tensor.matmul(
                            oaug[:],
                            ep[kb][:, c * P:(c + 1) * P],
                            cx.Vaug[kb0 + kb][:],
                            start=(kb == 0), stop=(kb == qb),
                        )
                    recip = cx.smallp.tile([P, 1], F32, tag="recip", name="recip")
                    nc.vector.reciprocal(recip[:], oaug[:, HEAD_DIM:HEAD_DIM + 1])
                    osb = cx.smallp.tile([P, P], BF16, tag="osb", name="osb")
                    nc.vector.tensor_scalar(osb[:], oaug[:, 0:HEAD_DIM], recip[:], None, mybir.AluOpType.mult)
                    ot = cx.pss.tile([P, P], BF16, tag="small", name="small")
                    nc.tensor.transpose(ot[:], osb[:], cx.identity[:])
                    ot_sb = cx.smallp.tile([P, P], BF16, tag="otsb", name="otsb")
                    nc.vector.tensor_copy(ot_sb[:], ot[:])
                    gq = b * QC_PER_BATCH + qb   # global q chunk 0..31
                    dst, toff = gq // 4, (gq % 4) * P
                    nc.sync.dma_start(
                        out=cx.a2a_in[dst * 256 + hq * P:dst * 256 + (hq + 1) * P, toff:toff + P],
                        in_=ot_sb[:],
                    )


def phase_oproj(cx):
    nc = cx.nc
    nc.gpsimd.collective_compute(
        "AllToAll",
        mybir.AluOpType.bypass,
        replica_groups=[list(range(N_CORES))],
        ins=[cx.a2a_in.opt()],
        outs=[cx.a2a_out.opt()],
    )
    attnT = [cx.attntp.tile([P, 512], BF16, tag="attnt", name="attnt") for _ in range(HS)]
    for fs in range(HS):
        nc.sync.dma_start(out=attnT[fs][:], in_=cx.a2a_out[fs * P:(fs + 1) * P, :])
    for hb in range(4):
        wo_sb = [cx.wop.tile([P, 512], BF16, tag="wo", name="wo") for _ in range(HS)]
        for fs in range(HS):
            nc.sync.dma_start(out=wo_sb[fs][:], in_=cx.io["woT"][fs * P:(fs + 1) * P, hb * 512:(hb + 1) * 512])
        for tb4 in range(4):
            o_ps = cx.psb.tile([P, 512], F32, tag="big", name="big")
            for fs in range(HS):
                nc.tensor.matmul(
                    o_ps[:], attnT[fs][:, tb4 * P:(tb4 + 1) * P], wo_sb[fs][:],
                    start=(fs == 0), stop=(fs == HS - 1),
                )
            o_sb = cx.outp.tile([P, 512], F32, tag="outsb", name="outsb")
            nc.vector.tensor_copy(o_sb[:], o_ps[:])
            nc.sync.dma_start(out=cx.io["out"][tb4 * P:(tb4 + 1) * P, hb * 512:(hb + 1) * 512], in_=o_sb[:])


def emit_program(nc, nreps=1):
    io = {
        "xT": nc.dram_tensor("xT", [HIDDEN, T], F32R, kind="ExternalInput"),
        "wqT": nc.dram_tensor("wqT", [HIDDEN, HEADS_PER_CORE * HEAD_DIM], F32R, kind="ExternalInput"),
        "wkT": nc.dram_tensor("wkT", [HIDDEN, HEAD_DIM], F32R, kind="ExternalInput"),
        "wvT": nc.dram_tensor("wvT", [HIDDEN, HEAD_DIM], F32R, kind="ExternalInput"),
        "woT": nc.dram_tensor("woT", [HIDDEN, HIDDEN], BF16, kind="ExternalInput"),
        "cosT": nc.dram_tensor("cosT", [P, S], F32, kind="ExternalInput"),
        "sinTs": nc.dram_tensor("sinTs", [P, S], F32, kind="ExternalInput"),
        "trimask": nc.dram_tensor("trimask", [P, P], BF16, kind="ExternalInput"),
        "out": nc.dram_tensor("out", [T // N_CORES, HIDDEN], F32, kind="ExternalOutput"),
    }
    with tile.TileContext(nc) as tc:
        for _rep in range(nreps):
            with ExitStack() as es:
                cx = Ctx(tc, es, io)
                phase_proj(cx)
                phase_attention(cx)
                phase_oproj(cx)


def build_program(nreps=1):
    nc = bacc.Bacc("TRN2", target_bir_lowering=False, debug=False, num_devices=N_CORES)
    emit_program(nc, nreps)
    nc.compile()
    return nc


def shard_inputs(x, Wq, Wk, Wv, Wo):
    x = np.asarray(x, dtype=np.float32)
    Wq = np.asarray(Wq, dtype=np.float32)
    Wk = np.asarray(Wk, dtype=np.float32)
    Wv = np.asarray(Wv, dtype=np.float32)
    Wo = np.asarray(Wo, dtype=np.float32)

    # x: [B,S,H] -> xT [H, B*S] (batch-major tokens)
    xT = np.ascontiguousarray(x.reshape(T, HIDDEN).T)
    woT = np.ascontiguousarray(Wo.T).astype(ml_dtypes.bfloat16)

    # RoPE tables in [d, t] layout, sin pre-signed for the pair-swap trick
    j = np.arange(0, HEAD_DIM, 2, dtype=np.float32)
    inv_freq = 1.0 / (ROPE_BASE ** (j / HEAD_DIM))           # [64]
    pos = np.arange(S, dtype=np.float32)
    ang = inv_freq[:, None] * pos[None, :]                   # [64, S]
    cosT = np.repeat(np.cos(ang), 2, axis=0).astype(np.float32)   # [128, S]
    sin = np.sin(ang)
    sinTs = np.empty((HEAD_DIM, S), np.float32)
    sinTs[0::2] = -sin
    sinTs[1::2] = sin

    trimask = np.triu(np.ones((P, P), np.float32)).astype(ml_dtypes.bfloat16)

    in_maps = []
    for i in range(N_CORES):
        g = i // 2
        in_maps.append({
            "xT": xT,
            "wqT": np.ascontiguousarray(Wq[2 * i * HEAD_DIM:(2 * i + 2) * HEAD_DIM, :].T),
            "wkT": np.ascontiguousarray(Wk[g * HEAD_DIM:(g + 1) * HEAD_DIM, :].T),
            "wvT": np.ascontiguousarray(Wv[g * HEAD_DIM:(g + 1) * HEAD_DIM, :].T),
            "woT": woT,
            "cosT": cosT,
            "sinTs": sinTs,
            "trimask": trimask,
        })
    return in_maps


_CACHED_NC = None


def kernel(x, Wq, Wk, Wv, Wo):
    global _CACHED_NC
    if _CACHED_NC is None:
        _CACHED_NC = build_program()
    nc = _CACHED_NC
    in_maps = shard_inputs(x, Wq, Wk, Wv, Wo)
    res = run_bass_kernel_spmd(nc, in_maps, core_ids=list(range(N_CORES)))
    outs = np.concatenate([res.results[i]["out"] for i in range(N_CORES)], axis=0)
    return outs.reshape(B, S, HIDDEN).astype(np.float32)


# revision 15
# speedup vs baseline: 8.4577x; 8.4577x over previous
"""Ernie4.5 attention layer on 8 Trainium2 NeuronCores.

Sharding (tensor-parallel over heads, 8-way):
  core i owns query heads {2i, 2i+1} and kv head i//2 (kv groups stay
  aligned with their query heads; kv projections are duplicated on the
  two cores sharing a kv head).
  Every core processes all B*S = 4096 tokens through projections + RoPE +
  attention for its own heads, producing attn^T [256 f, 4096 tok].
  An 8-core AllToAll redistributes head-slices -> token-slices, after
  which core i holds attn^T [2048 f, 512 tok] for global tokens
  [512*i, 512*(i+1)) and computes the o_proj rows for those tokens.
  Host-side gather is a pure concatenation.

Numerics: projections / scores run in float32r (tf32-like, ~1e-4 rel),
softmax probabilities and the PV / o_proj matmuls run in bf16 with fp32
PSUM accumulation.
"""

import math
from contextlib import ExitStack

import numpy as np
import ml_dtypes

import concourse.bass as bass
import concourse.tile as tile
from concourse import bacc, mybir
from concourse.bass_utils import run_bass_kernel_spmd
from concourse.masks import make_identity

HIDDEN = 2048
N_HEADS = 16
N_KV_HEADS = 4
HEAD_DIM = 128
ROPE_BASE = 10000.0
B, S = 2, 2048
T = B * S                    # 4096 global tokens (batch-major)
N_CORES = 8
HEADS_PER_CORE = 2
P = 128
SCALE = 1.0 / math.sqrt(HEAD_DIM)

F32 = mybir.dt.float32
F32R = mybir.dt.float32r
BF16 = mybir.dt.bfloat16

HS = HIDDEN // P             # 16 hidden slices
TB = T // 512                # 8 token blocks of 512
KB = T // P                  # 32 key blocks of 128 (16 per batch)
QC_PER_BATCH = S // P        # 16 query chunks of 128 per batch

SWAP_MASK = [i ^ 1 for i in range(32)]  # pair swap within 32-partition groups


class Ctx:
    """Pools + persistent tiles for one repetition of the program."""

    def __init__(self, tc, es, io):
        nc = tc.nc
        self.tc, self.nc, self.io = tc, nc, io
        pool = lambda name, bufs, **kw: es.enter_context(tc.tile_pool(name=name, bufs=bufs, **kw))
        self.const = pool("const", 1)
        self.wqp = pool("wq", 1)
        self.wkvp = pool("wkv", 1)
        self.xtp = pool("xt", 8)
        self.qtp = pool("qt", 1)
        self.ropep = pool("rope", 3)
        self.vaugp = pool("vaug", 1)
        self.expp = pool("expp", 9)          # [128,1024] bf16 pair tiles
        self.smallp = pool("small_sb", 4)
        self.attntp = pool("attnt", 16)
        self.wop = pool("wo", 16)
        self.outp = pool("outsb", 3)
        self.psb = pool("ps_big", 3, space="PSUM")    # [128,1024] = 2 banks each
        self.pss = pool("ps_small", 2, space="PSUM")  # 1 bank each
        self.dram = pool("dram", 1, space="DRAM")

        self.identity = self.const.tile([P, P], BF16, tag="identity", name="identity")
        make_identity(nc, self.identity[:])
        self.tri_sb = self.const.tile([P, P], BF16, tag="tri", name="tri")
        nc.sync.dma_start(out=self.tri_sb[:], in_=io["trimask"][:])
        self.cos_sb = self.const.tile([P, S], F32, tag="cos", name="cos")
        nc.sync.dma_start(out=self.cos_sb[:], in_=io["cosT"][:])
        self.sin_sb = self.const.tile([P, S], F32, tag="sin", name="sin")
        nc.sync.dma_start(out=self.sin_sb[:], in_=io["sinTs"][:])

        self.wq_sb = [self.wqp.tile([P, HEADS_PER_CORE * HEAD_DIM], F32R, tag=f"wq{h}", name=f"wq{h}") for h in range(HS)]
        self.wk_sb = [self.wkvp.tile([P, HEAD_DIM], F32R, tag=f"wk{h}", name=f"wk{h}") for h in range(HS)]
        self.wv_sb = [self.wkvp.tile([P, HEAD_DIM], F32R, tag=f"wv{h}", name=f"wv{h}") for h in range(HS)]
        for h in range(HS):
            nc.sync.dma_start(out=self.wq_sb[h][:], in_=io["wqT"][h * P:(h + 1) * P, :])
            nc.sync.dma_start(out=self.wk_sb[h][:], in_=io["wkT"][h * P:(h + 1) * P, :])
            nc.sync.dma_start(out=self.wv_sb[h][:], in_=io["wvT"][h * P:(h + 1) * P, :])

        # persistent rotated Q^T (per head) and K^T, all [128, 4096] f32r
        self.QT = [self.qtp.tile([P, T], F32R, tag=f"QT{i}", name=f"QT{i}") for i in range(HEADS_PER_CORE)]
        self.KT = self.qtp.tile([P, T], F32R, tag="KT", name="KT")
        # V_aug[kb]: [128 tok, 128 d + ones column] bf16
        self.Vaug = [self.vaugp.tile([P, HEAD_DIM + 1], BF16, tag=f"va{k}", name=f"va{k}") for k in range(KB)]

        # per-head a2a buffers: [8 dst cores x 128 f, 512 tok] bf16
        self.a2a_in = [self.dram.tile([N_CORES * P, 512], BF16, tag=f"a2a_in{h}", name=f"a2a_in{h}")
                       for h in range(HEADS_PER_CORE)]
        self.a2a_out = [self.dram.tile([N_CORES * P, 512], BF16, tag=f"a2a_out{h}", name=f"a2a_out{h}")
                        for h in range(HEADS_PER_CORE)]


def _rope(cx, ps, dst, tb):
    """dst = psum * cos + pairswap(psum) * signed_sin, written as f32r."""
    nc = cx.nc
    s0 = (tb % (S // 512)) * 512  # seq offset within batch
    shuf = cx.ropep.tile([P, 512], F32, tag="shuf", name="shuf")
    nc.vector.stream_shuffle(shuf[:], ps, SWAP_MASK)
    t1 = cx.ropep.tile([P, 512], F32, tag="t1", name="t1")
    nc.vector.tensor_mul(t1[:], ps, cx.cos_sb[:, s0:s0 + 512])
    t2 = cx.ropep.tile([P, 512], F32, tag="t2", name="t2")
    nc.vector.tensor_mul(t2[:], shuf[:], cx.sin_sb[:, s0:s0 + 512])
    nc.vector.tensor_add(dst, t1[:], t2[:])


def phase_proj(cx):
    nc = cx.nc
    for tb in range(TB):
        xt = [cx.xtp.tile([P, 512], F32R, tag="xt", name="xt") for _ in range(HS)]
        for h in range(HS):
            nc.sync.dma_start(out=xt[h][:], in_=cx.io["xT"][h * P:(h + 1) * P, tb * 512:(tb + 1) * 512])
        qq_ps = cx.psb.tile([P, 1024], F32, tag="big", name="big")   # Q0 | Q1
        kv_ps = cx.psb.tile([P, 1024], F32, tag="big", name="big")   # K | V^T
        for h in range(HS):
            st, sp = h == 0, h == HS - 1
            nc.tensor.matmul(qq_ps[:, 0:512], cx.wq_sb[h][:, 0:P], xt[h][:], start=st, stop=sp)
            nc.tensor.matmul(qq_ps[:, 512:1024], cx.wq_sb[h][:, P:2 * P], xt[h][:], start=st, stop=sp)
            nc.tensor.matmul(kv_ps[:, 0:512], cx.wk_sb[h][:], xt[h][:], start=st, stop=sp)
            nc.tensor.matmul(kv_ps[:, 512:1024], cx.wv_sb[h][:], xt[h][:], start=st, stop=sp)
        _rope(cx, qq_ps[:, 0:512], cx.QT[0][:, tb * 512:(tb + 1) * 512], tb)
        _rope(cx, qq_ps[:, 512:1024], cx.QT[1][:, tb * 512:(tb + 1) * 512], tb)
        _rope(cx, kv_ps[:, 0:512], cx.KT[:, tb * 512:(tb + 1) * 512], tb)
        # V: copy to bf16, transpose 128x128 blocks into Vaug ([tok, d])
        vt_sb = cx.smallp.tile([P, 512], BF16, tag="vtsb", name="vtsb")
        nc.vector.tensor_copy(vt_sb[:], kv_ps[:, 512:1024])
        for j in range(4):
            kbi = tb * 4 + j
            vtt = cx.pss.tile([P, P], BF16, tag="small", name="small")
            nc.tensor.transpose(vtt[:], vt_sb[:, j * P:(j + 1) * P], cx.identity[:])
            nc.vector.tensor_copy(cx.Vaug[kbi][:, 0:HEAD_DIM], vtt[:])
            nc.vector.memset(cx.Vaug[kbi][:, HEAD_DIM:HEAD_DIM + 1], 1.0)


def phase_attention(cx, hq):
    nc = cx.nc
    if True:
        for b in range(B):
            kb0 = b * (S // P)      # first key block of this batch
            q0 = b * S              # first token of this batch
            for qt in range(S // 512):
                i0 = qt * 4         # first q chunk (of 128) in this tile
                ep = []
                for kb in range(i0 + 4):
                    m = max(kb - i0, 0)  # leading q-chunks of this tile never read
                    st_ps = cx.psb.tile([P, 512], F32, tag="big", name="big")
                    nc.tensor.matmul(
                        st_ps[:, m * P:512],
                        cx.KT[:, (kb0 + kb) * P:(kb0 + kb + 1) * P],
                        cx.QT[hq][:, q0 + qt * 512 + m * P:q0 + (qt + 1) * 512],
                        start=True, stop=True,
                    )
                    e = cx.expp.tile([P, 512], BF16, tag="expp", name="expp")
                    nc.scalar.activation(e[:, m * P:512], st_ps[:, m * P:512], mybir.ActivationFunctionType.Exp, scale=SCALE)
                    if kb >= i0:
                        nc.vector.tensor_mul(e[:, m * P:(m + 1) * P], e[:, m * P:(m + 1) * P], cx.tri_sb[:])
                    ep.append(e)
                for c in range(4):
                    qb = i0 + c
                    oaug = cx.pss.tile([P, HEAD_DIM + 1], F32, tag="small", name="small")
                    for kb in range(qb + 1):
                        et, off = ep[kb]
                        nc.tensor.matmul(
                            oaug[:],
                            et[:, off + c * P:off + (c + 1) * P],
                            cx.Vaug[kb0 + kb][:],
                            start=(kb == 0), stop=(kb == qb),
                        )
                    recip = cx.smallp.tile([P, 1], F32, tag="recip", name="recip")
                    nc.vector.reciprocal(recip[:], oaug[:, HEAD_DIM:HEAD_DIM + 1])
                    osb = cx.smallp.tile([P, P], BF16, tag="osb", name="osb")
                    nc.vector.tensor_scalar(osb[:], oaug[:, 0:HEAD_DIM], recip[:], None, mybir.AluOpType.mult)
                    ot = cx.pss.tile([P, P], BF16, tag="small", name="small")
                    nc.tensor.transpose(ot[:], osb[:], cx.identity[:])
                    ot_sb = cx.smallp.tile([P, P], BF16, tag="otsb", name="otsb")
                    nc.vector.tensor_copy(ot_sb[:], ot[:])
                    gq = b * QC_PER_BATCH + qb   # global q chunk 0..31
                    dst, toff = gq // 4, (gq % 4) * P
                    nc.sync.dma_start(
                        out=cx.a2a_in[hq][dst * P:(dst + 1) * P, toff:toff + P],
                        in_=ot_sb[:],
                    )


def a2a(cx, hq):
    cx.nc.gpsimd.collective_compute(
        "AllToAll",
        mybir.AluOpType.bypass,
        replica_groups=[list(range(N_CORES))],
        ins=[cx.a2a_in[hq].opt()],
        outs=[cx.a2a_out[hq].opt()],
    )


def phase_oproj(cx):
    nc = cx.nc
    attnT = []
    for fs in range(HS):
        t = cx.attntp.tile([P, 512], BF16, tag="attnt", name="attnt")
        nc.sync.dma_start(out=t[:], in_=cx.a2a_out[fs % 2][(fs // 2) * P:(fs // 2 + 1) * P, :])
        attnT.append(t)
    order = [fs for fs in range(HS) if fs % 2 == 0] + [fs for fs in range(HS) if fs % 2 == 1]
    for hb in range(4):
        wo_sb = [cx.wop.tile([P, 512], BF16, tag="wo", name="wo") for _ in range(HS)]
        for fs in range(HS):
            nc.gpsimd.dma_start(out=wo_sb[fs][:], in_=cx.io["woT"][fs * P:(fs + 1) * P, hb * 512:(hb + 1) * 512])
        for tb4 in range(4):
            o_ps = cx.psb.tile([P, 512], F32, tag="big", name="big")
            for n, fs in enumerate(order):
                nc.tensor.matmul(
                    o_ps[:], attnT[fs][:, tb4 * P:(tb4 + 1) * P], wo_sb[fs][:],
                    start=(n == 0), stop=(n == HS - 1),
                )
            o_sb = cx.outp.tile([P, 512], F32, tag="outsb", name="outsb")
            nc.vector.tensor_copy(o_sb[:], o_ps[:])
            nc.sync.dma_start(out=cx.io["out"][tb4 * P:(tb4 + 1) * P, hb * 512:(hb + 1) * 512], in_=o_sb[:])


def emit_program(nc, nreps=1):
    io = {
        "xT": nc.dram_tensor("xT", [HIDDEN, T], F32R, kind="ExternalInput"),
        "wqT": nc.dram_tensor("wqT", [HIDDEN, HEADS_PER_CORE * HEAD_DIM], F32R, kind="ExternalInput"),
        "wkT": nc.dram_tensor("wkT", [HIDDEN, HEAD_DIM], F32R, kind="ExternalInput"),
        "wvT": nc.dram_tensor("wvT", [HIDDEN, HEAD_DIM], F32R, kind="ExternalInput"),
        "woT": nc.dram_tensor("woT", [HIDDEN, HIDDEN], BF16, kind="ExternalInput"),
        "cosT": nc.dram_tensor("cosT", [P, S], F32, kind="ExternalInput"),
        "sinTs": nc.dram_tensor("sinTs", [P, S], F32, kind="ExternalInput"),
        "trimask": nc.dram_tensor("trimask", [P, P], BF16, kind="ExternalInput"),
        "out": nc.dram_tensor("out", [T // N_CORES, HIDDEN], F32, kind="ExternalOutput"),
    }
    with tile.TileContext(nc) as tc:
        for _rep in range(nreps):
            with ExitStack() as es:
                cx = Ctx(tc, es, io)
                phase_proj(cx)
                for hq in range(HEADS_PER_CORE):
                    phase_attention(cx, hq)
                    a2a(cx, hq)
                phase_oproj(cx)


def build_program(nreps=1):
    nc = bacc.Bacc("TRN2", target_bir_lowering=False, debug=False, num_devices=N_CORES)
    emit_program(nc, nreps)
    nc.compile()
    return nc


def shard_inputs(x, Wq, Wk, Wv, Wo):
    x = np.asarray(x, dtype=np.float32)
    Wq = np.asarray(Wq, dtype=np.float32)
    Wk = np.asarray(Wk, dtype=np.float32)
    Wv = np.asarray(Wv, dtype=np.float32)
    Wo = np.asarray(Wo, dtype=np.float32)

    # x: [B,S,H] -> xT [H, B*S] (batch-major tokens)
    xT = np.ascontiguousarray(x.reshape(T, HIDDEN).T)
    woT = np.ascontiguousarray(Wo.T).astype(ml_dtypes.bfloat16)

    # RoPE tables in [d, t] layout, sin pre-signed for the pair-swap trick
    j = np.arange(0, HEAD_DIM, 2, dtype=np.float32)
    inv_freq = 1.0 / (ROPE_BASE ** (j / HEAD_DIM))           # [64]
    pos = np.arange(S, dtype=np.float32)
    ang = inv_freq[:, None] * pos[None, :]                   # [64, S]
    cosT = np.repeat(np.cos(ang), 2, axis=0).astype(np.float32)   # [128, S]
    sin = np.sin(ang)
    sinTs = np.empty((HEAD_DIM, S), np.float32)
    sinTs[0::2] = -sin
    sinTs[1::2] = sin

    trimask = np.triu(np.ones((P, P), np.float32)).astype(ml_dtypes.bfloat16)

    in_maps = []
    for i in range(N_CORES):
        g = i // 2
        in_maps.append({
            "xT": xT,
            "wqT": np.ascontiguousarray(Wq[2 * i * HEAD_DIM:(2 * i + 2) * HEAD_DIM, :].T),
            "wkT": np.ascontiguousarray(Wk[g * HEAD_DIM:(g + 1) * HEAD_DIM, :].T),
            "wvT": np.ascontiguousarray(Wv[g * HEAD_DIM:(g + 1) * HEAD_DIM, :].T),
            "woT": woT,
            "cosT": cosT,
            "sinTs": sinTs,
            "trimask": trimask,
        })
    return in_maps


_CACHED_NC = None


def kernel(x, Wq, Wk, Wv, Wo):
    global _CACHED_NC
    if _CACHED_NC is None:
        _CACHED_NC = build_program()
    nc = _CACHED_NC
    in_maps = shard_inputs(x, Wq, Wk, Wv, Wo)
    res = run_bass_kernel_spmd(nc, in_maps, core_ids=list(range(N_CORES)))
    outs = np.concatenate([res.results[i]["out"] for i in range(N_CORES)], axis=0)
    return outs.reshape(B, S, HIDDEN).astype(np.float32)
